# revision 20
# baseline (speedup 1.0000x reference)
"""Trainium2 Bass kernel for a GPT-J-style (parallel-residual) decoder layer.

Problem: B=2, S=2048, D=1024, H=16 heads x 64, rotary_dim=16, FF=4096, causal.

Sharding (8 NeuronCores): data-parallel over batch (2) x tensor-parallel over
heads/FFN (4).  Core c handles batch c//4 and TP rank r=c%4: heads 4r..4r+3
(256 of the 1024 attention dims), FFN rows 1024r..1024r+1024.
LayerNorm affine params are folded into the weights on the host, so the device
computes a single normalized activation xhat shared by attention and FFN.
Each core returns partial^T = (attn_partial + ffn_partial)^T in [D, S]; the
host sums the 4 TP partials per batch and adds x + b_o + b2.
"""

import numpy as np
import ml_dtypes

import concourse.bass as bass
import concourse.mybir as mybir
import concourse.tile as tile
import concourse.bass_utils as bass_utils
from concourse import bacc
from concourse.bass import ds, ts

B, S, D = 2, 2048, 1024
H, HD = 16, 64
ROT, RH = 16, 8
FF = 4096
EPS = 1e-5
P = 128
NT = S // P            # 16 sequence tiles
DC = D // P            # 8 model-dim chunks
NH = 4                 # heads per core
DSH = NH * HD          # 256 attention dims per core
FSH = FF // 4          # 1024 FFN rows per core
NCORES = 8

F32 = mybir.dt.float32
BF16 = mybir.dt.bfloat16
AF = mybir.ActivationFunctionType
ALU = mybir.AluOpType
bf16 = ml_dtypes.bfloat16


def _body(tc, aps, gelu_func):
    nc = tc.nc
    x_d = aps["x"]
    wqkv_d = aps["wqkv"]
    bqkv_d = aps["bqkv"]
    wo_d = aps["wo"]
    w1_d = aps["w1"]
    b1_d = aps["b1p"]
    w2_d = aps["w2"]
    cos_d = aps["cosr"]
    sin_d = aps["sinr"]
    mask_d = aps["maskd"]
    out_d = aps["outp"].rearrange("(c p) s -> c p s", p=P)   # [8, 128, 2048]

    with (
        tc.tile_pool(name="const", bufs=1) as const,
        tc.tile_pool(name="big", bufs=1) as big,
        tc.tile_pool(name="xp", bufs=2) as xp,
        tc.tile_pool(name="statp", bufs=4) as statp,
        tc.tile_pool(name="xhp", bufs=3) as xhp,
        tc.tile_pool(name="rotp", bufs=3) as rotp,
        tc.tile_pool(name="wstp", bufs=5) as wstp,
        tc.tile_pool(name="ptp", bufs=4) as ptp,
        tc.tile_pool(name="sump", bufs=4) as sump,
        tc.tile_pool(name="obp", bufs=3) as obp,
    ):
        # ---- persistent SBUF ----
        wqkv_sb = const.tile([P, DC, 3 * DSH], BF16)
        nc.sync.dma_start(wqkv_sb[:], wqkv_d)
        bqkv_sb = const.tile([P, 3 * DSH], F32)
        nc.sync.dma_start(bqkv_sb[:], bqkv_d)
        wo_sb = const.tile([P, 2, D], BF16)
        nc.sync.dma_start(wo_sb[:], wo_d)
        b1_sb = const.tile([P, DC], F32)
        nc.sync.dma_start(b1_sb[:], b1_d)
        cos_sb = const.tile([P, NT, RH], BF16)
        nc.sync.dma_start(cos_sb[:], cos_d)
        sin_sb = const.tile([P, NT, RH], BF16)
        nc.sync.dma_start(sin_sb[:], sin_d)
        mask_sb = const.tile([P, P], BF16)
        nc.sync.dma_start(mask_sb[:], mask_d)
        eps_sb = const.tile([P, 1], F32)
        nc.vector.memset(eps_sb[:], EPS)
        ones_sb = const.tile([1, HD], BF16)
        nc.vector.memset(ones_sb[:], 1.0)

        xhatT = [big.tile([P, DC, S // 2], BF16, name=f"xhatT{i}")
                 for i in range(2)]          # xhat transposed [d, s], s-halves
        qk = big.tile([P, NT, 2 * DSH], BF16)       # q,k token-major
        vp = big.tile([P, NT, NH, HD + 2], BF16)    # v token-major + ones col
        qe = big.tile([P, 2, S], BF16)              # q e-major
        ke = big.tile([P, 2, S], BF16)              # k e-major
        ot = big.tile([P, 2, S], BF16)              # attn out (normalized), e-major
        hid = big.tile([P, DC, S], BF16)            # ffn hidden, f-major

        nc.vector.memset(vp[:, :, :, HD:HD + 1], 1.0)

        # DRAM staging for the big bf16 transposes (token-major -> dim-major)
        stg = tc.alloc_tile_pool(name="stg", bufs=1, space="DRAM")
        xh_dram = stg.tile([S, D], BF16)
        qk_dram = stg.tile([S, 2 * DSH], BF16)
        ff2_dram = stg.tile([D, S], BF16)

        # ---- Stage A: LayerNorm stats + xhat (bf16), stage via DRAM ----
        for t in range(NT):
            x_t = xp.tile([P, D], F32, tag="xt")
            nc.sync.dma_start(x_t[:], x_d[ts(t, P), :])
            st = statp.tile([P, 2, 6], F32, tag="st")
            xr = x_t[:].rearrange("p (a f) -> p a f", f=512)
            for sg in range(2):
                nc.vector.bn_stats(st[:, sg, :], xr[:, sg, :])
            mv = statp.tile([P, 2], F32, tag="mv")
            nc.vector.bn_aggr(mv[:], st[:])
            std = statp.tile([P, 1], F32, tag="sd")
            nc.scalar.activation(std[:], mv[:, 1:2], AF.Sqrt, bias=eps_sb[:])
            rstd = statp.tile([P, 1], F32, tag="rs")
            nc.vector.reciprocal(rstd[:], std[:])
            mneg = statp.tile([P, 1], F32, tag="mn")
            nc.vector.scalar_tensor_tensor(
                out=mneg[:], in0=mv[:, 0:1], scalar=-1.0, in1=rstd[:],
                op0=ALU.mult, op1=ALU.mult)
            xh = xhp.tile([P, D], BF16, tag="xh")
            nc.scalar.activation(xh[:], x_t[:], AF.Identity,
                                 bias=mneg[:], scale=rstd[:])
            nc.sync.dma_start(xh_dram[ts(t, P), :], xh[:])
            if t % (NT // 2) == NT // 2 - 1:
                half = t // (NT // 2)
                for c in range(DC):
                    nc.sync.dma_start_transpose(
                        xhatT[half][:, c, :],
                        xh_dram[ds(half * (S // 2), S // 2), ds(P * c, P)])

        # ---- Stage B: QKV projection + FFN-1/GELU (one PE-dense phase;
        # GELU runs here so the ACT table never alternates with Exp) ----
        with (
            tc.tile_pool(name="qaps", bufs=2, space="PSUM") as qaps,
            tc.tile_pool(name="qbps", bufs=2, space="PSUM") as qbps,
            tc.tile_pool(name="ff1ps", bufs=4, space="PSUM") as ff1ps,
        ):
            for t in range(NT):
                psa = qaps.tile([P, 512], F32, tag="qa")
                psb = qbps.tile([P, 256], F32, tag="qb")
                for c in range(DC):
                    l = xhatT[t // (NT // 2)][:, c, ts(t % (NT // 2), P)]
                    nc.tensor.matmul(psa[:], lhsT=l, rhs=wqkv_sb[:, c, 0:512],
                                     start=(c == 0), stop=(c == DC - 1))
                    nc.tensor.matmul(psb[:], lhsT=l, rhs=wqkv_sb[:, c, 512:768],
                                     start=(c == 0), stop=(c == DC - 1))
                nc.vector.tensor_tensor(out=qk[:, t, :], in0=psa[:],
                                        in1=bqkv_sb[:, 0:512], op=ALU.add)
                nc.vector.tensor_tensor(
                    out=vp[:, t, :, 0:HD],
                    in0=psb[:].rearrange("p (h e) -> p h e", h=NH),
                    in1=bqkv_sb[:, 512:768].rearrange("p (h e) -> p h e", h=NH),
                    op=ALU.add)

            # rotary on q and k (token-major, in place), per s-half so the
            # staging stores can start before the second half finishes
            HT = NT // 2
            for hf in range(2):
                cosb = cos_sb[:, ds(HT * hf, HT), :].unsqueeze(2) \
                    .to_broadcast([P, HT, NH, RH])
                sinb = sin_sb[:, ds(HT * hf, HT), :].unsqueeze(2) \
                    .to_broadcast([P, HT, NH, RH])
                for part in range(2):   # 0: q, 1: k
                    sl = qk[:, ds(HT * hf, HT), ds(DSH * part, DSH)].rearrange(
                        "p t (h e) -> p t h e", h=NH)
                    x1 = sl[:, :, :, 0:RH]
                    x2 = sl[:, :, :, RH:ROT]
                    t1 = rotp.tile([P, HT, NH, RH], BF16, tag="rt")
                    t2 = rotp.tile([P, HT, NH, RH], BF16, tag="rt")
                    t3 = rotp.tile([P, HT, NH, RH], BF16, tag="rt")
                    nc.vector.tensor_tensor(out=t1[:], in0=x1, in1=cosb,
                                            op=ALU.mult)
                    nc.vector.tensor_tensor(out=t2[:], in0=x2, in1=sinb,
                                            op=ALU.mult)
                    nc.vector.tensor_tensor(out=t1[:], in0=t1[:], in1=t2[:],
                                            op=ALU.subtract)
                    nc.vector.tensor_tensor(out=t2[:], in0=x1, in1=sinb,
                                            op=ALU.mult)
                    nc.vector.tensor_tensor(out=t3[:], in0=x2, in1=cosb,
                                            op=ALU.mult)
                    nc.vector.tensor_tensor(out=t2[:], in0=t2[:], in1=t3[:],
                                            op=ALU.add)
                    nc.vector.tensor_copy(out=x1, in_=t1[:])
                    nc.vector.tensor_copy(out=x2, in_=t2[:])
                for t in range(HT * hf, HT * hf + HT):
                    eng = nc.sync if t % 2 == 0 else nc.scalar
                    eng.dma_start(qk_dram[ts(t, P), :], qk[:, t, :])
            for c in range(2):
                nc.sync.dma_start_transpose(qe[:, c, :],
                                            qk_dram[:, ds(P * c, P)])
                nc.sync.dma_start_transpose(ke[:, c, :],
                                            qk_dram[:, ds(DSH + P * c, P)])

            # FFN first matmul + GELU
            for ft in range(DC):
                w1t = wstp.tile([P, DC, P], BF16, tag="wst", name=f"w1t_{ft}")
                nc.scalar.dma_start(w1t[:], w1_d[:, ft])
                pss = [ff1ps.tile([P, 512], F32, tag="ff1",
                                  name=f"ff1_{ft}_{i}") for i in range(4)]
                for c in range(DC):
                    for sc in range(4):
                        nc.tensor.matmul(
                            pss[sc][:], lhsT=w1t[:, c, :],
                            rhs=xhatT[sc // 2][:, c, ds(512 * (sc % 2), 512)],
                            start=(c == 0), stop=(c == DC - 1))
                for sc in range(4):
                    nc.scalar.activation(hid[:, ft, ds(512 * sc, 512)],
                                         pss[sc][:], gelu_func,
                                         bias=b1_sb[:, ft:ft + 1])

        # ---- Stage C: attention (ACT-bound exp) interleaved with FFN-2
        # matmuls (pure PE+DVE, partials staged to DRAM in bf16) ----
        ff2_deck = [[0, 1], [2, 3], [4, 5], [6, 7], []]
        with (
            tc.tile_pool(name="scps", bufs=3, space="PSUM") as scps,
            tc.tile_pool(name="ovps", bufs=4, space="PSUM") as ovps,
            tc.tile_pool(name="f2ps", bufs=1, space="PSUM") as f2ps,
        ):
            def emit_ff2(et):
                w2t = wstp.tile([P, DC, P], BF16, tag="wst", name=f"w2t_{et}")
                nc.scalar.dma_start(w2t[:], w2_d[:, et])
                for sc in range(4):
                    ps = f2ps.tile([P, 512], F32, tag="f2",
                                   name=f"f2_{et}_{sc}")
                    for c in range(DC):
                        nc.tensor.matmul(ps[:], lhsT=w2t[:, c, :],
                                         rhs=hid[:, c, ds(512 * sc, 512)],
                                         start=(c == 0), stop=(c == DC - 1))
                    ob2 = obp.tile([P, 512], BF16, tag="ob2")
                    nc.vector.tensor_copy(out=ob2[:], in_=ps[:])
                    nc.sync.dma_start(
                        ff2_dram[ds(P * et, P), ds(512 * sc, 512)], ob2[:])

            def emit_head(h):
                base = HD * (h % 2)
                cix = h // 2
                ov = [ovps.tile([HD + 1, 512], F32, tag="ov",
                                name=f"ov_{h}_{i}") for i in range(4)]
                for i in range(NT):
                    ncols = S - P * i
                    pt = ptp.tile([P, S], BF16, tag="pt")
                    nb = (ncols + 511) // 512
                    for bi in range(nb):
                        w = min(512, ncols - 512 * bi)
                        ps = scps.tile([P, 512], F32, tag="sc")
                        nc.tensor.matmul(
                            ps[:, :w],
                            lhsT=ke[base:base + HD, cix, ts(i, P)],
                            rhs=qe[base:base + HD, cix, ds(P * i + 512 * bi, w)],
                            start=True, stop=True)
                        nc.scalar.activation(pt[:, ds(512 * bi, w)], ps[:, :w],
                                             AF.Exp, scale=0.125)
                    nc.vector.tensor_tensor(out=pt[:, 0:P], in0=pt[:, 0:P],
                                            in1=mask_sb[:], op=ALU.mult)
                    for sc in range(i // 4, 4):
                        lo = max(512 * sc, P * i)
                        wid = 512 * (sc + 1) - lo
                        nc.tensor.matmul(
                            ov[sc][:, ds(lo - 512 * sc, wid)],
                            lhsT=vp[:, i, h, 0:HD + 1],
                            rhs=pt[:, ds(lo - P * i, wid)],
                            start=(i == 0), stop=(i == min(NT - 1, 4 * sc + 3)))
                rinvs = []
                for sc in range(4):
                    dst = ot[base:base + HD, cix, ds(512 * sc, 512)]
                    nc.vector.tensor_copy(out=dst, in_=ov[sc][0:HD, :])
                    sume = sump.tile([1, 512], F32, tag="se",
                                     name=f"se_{h}_{sc}")
                    nc.vector.tensor_copy(out=sume[:], in_=ov[sc][HD:HD + 1, :])
                    rinv = sump.tile([1, 512], F32, tag="ri",
                                     name=f"ri_{h}_{sc}")
                    nc.vector.reciprocal_approx_fast(out=rinv[:], in_=sume[:])
                    rinv_bf = sump.tile([1, 512], BF16, tag="rib",
                                        name=f"rib_{h}_{sc}")
                    nc.vector.tensor_copy(out=rinv_bf[:], in_=rinv[:])
                    rinvs.append(rinv_bf)
                for sc in range(4):
                    rbp = ovps.tile([HD + 1, 512], F32, tag="ov",
                                    name=f"rb_{h}_{sc}")
                    nc.tensor.matmul(rbp[0:HD, :], lhsT=ones_sb[:],
                                     rhs=rinvs[sc][:], start=True, stop=True)
                    dst = ot[base:base + HD, cix, ds(512 * sc, 512)]
                    nc.vector.tensor_tensor(out=dst, in0=dst, in1=rbp[0:HD, :],
                                            op=ALU.mult)

            for h in range(NH):
                emit_head(h)
                for et in ff2_deck[h]:
                    emit_ff2(et)

        # ---- Stage D: attn-out projection + add staged FFN-2 partials ----
        with tc.tile_pool(name="wops", bufs=4, space="PSUM") as wops:
            for et in range(DC):
                for sc in range(4):
                    po = wops.tile([P, 512], F32, tag="wo")
                    for c in range(2):
                        nc.tensor.matmul(po[:], lhsT=wo_sb[:, c, ts(et, P)],
                                         rhs=ot[:, c, ds(512 * sc, 512)],
                                         start=(c == 0), stop=(c == 1))
                    f2b = obp.tile([P, 512], BF16, tag="f2b")
                    nc.scalar.dma_start(
                        f2b[:], ff2_dram[ds(P * et, P), ds(512 * sc, 512)])
                    ob = obp.tile([P, 512], F32, tag="ob")
                    nc.vector.tensor_tensor(out=ob[:], in0=po[:], in1=f2b[:],
                                            op=ALU.add)
                    nc.sync.dma_start(out_d[et][:, ds(512 * sc, 512)], ob[:])
        stg.release()


def build(gelu_func=None):
    if gelu_func is None:
        gelu_func = AF.Gelu
    nc = bacc.Bacc("TRN2", target_bir_lowering=False, debug=False,
                   enable_asserts=True, num_devices=NCORES)
    aps = {}

    def din(name, shape, dtype):
        aps[name] = nc.dram_tensor(name, list(shape), dtype,
                                   kind="ExternalInput").ap()

    din("x", (S, D), F32)
    din("wqkv", (P, DC, 3 * DSH), BF16)
    din("bqkv", (P, 3 * DSH), F32)
    din("wo", (P, 2, D), BF16)
    din("w1", (P, DC, DC, P), BF16)
    din("b1p", (P, DC), F32)
    din("w2", (P, DC, DC, P), BF16)
    din("cosr", (P, NT, RH), BF16)
    din("sinr", (P, NT, RH), BF16)
    din("maskd", (P, P), BF16)
    aps["outp"] = nc.dram_tensor("outp", [D, S], F32,
                                 kind="ExternalOutput").ap()

    with tile.TileContext(nc) as tc:
        _body(tc, aps, gelu_func)
    nc.compile()
    return nc


def make_in_maps(inputs):
    x = np.asarray(inputs["x"], np.float32)
    Wqkv = np.asarray(inputs["W_qkv"], np.float32)
    b_qkv = np.asarray(inputs["b_qkv"], np.float32)
    Wo = np.asarray(inputs["W_o"], np.float32)
    ln1w = np.asarray(inputs["ln1_w"], np.float32)
    ln1b = np.asarray(inputs["ln1_b"], np.float32)
    ln2w = np.asarray(inputs["ln2_w"], np.float32)
    ln2b = np.asarray(inputs["ln2_b"], np.float32)
    W1 = np.asarray(inputs["W1"], np.float32)
    b1 = np.asarray(inputs["b1"], np.float32)
    W2 = np.asarray(inputs["W2"], np.float32)
    freqs = np.asarray(inputs["freqs_cis"], np.float32)

    cos = freqs[0, 0, :, :, 0]
    sin = freqs[0, 0, :, :, 1]
    cosr = np.ascontiguousarray(
        cos.reshape(NT, P, RH).transpose(1, 0, 2)).astype(bf16)
    sinr = np.ascontiguousarray(
        sin.reshape(NT, P, RH).transpose(1, 0, 2)).astype(bf16)
    kq = np.arange(P)
    maskd = (kq[:, None] <= kq[None, :]).astype(bf16)

    in_maps = []
    for core in range(NCORES):
        b = core // 4
        r = core % 4
        sl = slice(256 * r, 256 * r + 256)
        Ws = np.concatenate([Wqkv[0:D][sl], Wqkv[D:2 * D][sl],
                             Wqkv[2 * D:3 * D][sl]], 0)          # [768, 1024]
        bq = np.concatenate([b_qkv[0:D][sl], b_qkv[D:2 * D][sl],
                             b_qkv[2 * D:3 * D][sl]], 0)
        Wsp = Ws * ln1w[None, :]
        bqp = (bq + Ws @ ln1b).astype(np.float32)
        wqkv_l = np.ascontiguousarray(
            Wsp.T.reshape(DC, P, 3 * DSH).transpose(1, 0, 2)).astype(bf16)
        bqkv_l = np.ascontiguousarray(
            np.broadcast_to(bqp[None, :], (P, 3 * DSH))).astype(np.float32)
        Wos = Wo[:, sl]                                           # [1024, 256]
        wo_l = np.ascontiguousarray(
            Wos.T.reshape(2, P, D).transpose(1, 0, 2)).astype(bf16)
        W1s = W1[FSH * r: FSH * (r + 1)]                          # [1024, 1024]
        W1p = W1s * ln2w[None, :]
        b1p = (b1[FSH * r: FSH * (r + 1)] + W1s @ ln2b).astype(np.float32)
        w1_l = np.ascontiguousarray(
            W1p.reshape(DC, P, DC, P).transpose(3, 0, 2, 1)).astype(bf16)
        b1_l = np.ascontiguousarray(b1p.reshape(DC, P).T).astype(np.float32)
        W2s = W2[:, FSH * r: FSH * (r + 1)]                       # [1024, 1024]
        w2_l = np.ascontiguousarray(
            W2s.reshape(DC, P, DC, P).transpose(3, 0, 2, 1)).astype(bf16)
        in_maps.append(dict(
            x=np.ascontiguousarray(x[b]), wqkv=wqkv_l, bqkv=bqkv_l, wo=wo_l,
            w1=w1_l, b1p=b1_l, w2=w2_l, cosr=cosr, sinr=sinr, maskd=maskd))
    return in_maps


def gather(inputs, results):
    x = np.asarray(inputs["x"], np.float32)
    bias = (np.asarray(inputs["b_o"], np.float32)
            + np.asarray(inputs["b2"], np.float32))
    outs = [np.asarray(res["outp"], np.float32) for res in results]
    out = np.empty((B, S, D), np.float32)
    for b in range(B):
        acc = outs[4 * b] + outs[4 * b + 1] + outs[4 * b + 2] + outs[4 * b + 3]
        out[b] = x[b] + acc.T + bias[None, :]
    return out


_CACHE = {}


def kernel(**inputs):
    if "nc" not in _CACHE:
        _CACHE["nc"] = build()
    nc = _CACHE["nc"]
    in_maps = make_in_maps(inputs)
    res = bass_utils.run_bass_kernel_spmd(nc, in_maps,
                                          core_ids=list(range(NCORES)))
    return gather(inputs, res.results)


# revision 22
# speedup vs baseline: 1.0911x; 1.0911x over previous
"""Trainium2 Bass kernel for a GPT-J-style (parallel-residual) decoder layer.

Problem: B=2, S=2048, D=1024, H=16 heads x 64, rotary_dim=16, FF=4096, causal.

Sharding (8 NeuronCores): data-parallel over batch (2) x tensor-parallel over
heads/FFN (4).  Core c handles batch c//4 and TP rank r=c%4: heads 4r..4r+3
(256 of the 1024 attention dims), FFN rows 1024r..1024r+1024.
LayerNorm affine params are folded into the weights on the host, so the device
computes a single normalized activation xhat shared by attention and FFN.
Each core returns partial^T = (attn_partial + ffn_partial)^T in [D, S]; the
host sums the 4 TP partials per batch and adds x + b_o + b2.
"""

import numpy as np
import ml_dtypes

import concourse.bass as bass
import concourse.mybir as mybir
import concourse.tile as tile
import concourse.bass_utils as bass_utils
from concourse import bacc
from concourse.bass import ds, ts

B, S, D = 2, 2048, 1024
H, HD = 16, 64
ROT, RH = 16, 8
FF = 4096
EPS = 1e-5
P = 128
NT = S // P            # 16 sequence tiles
DC = D // P            # 8 model-dim chunks
NH = 4                 # heads per core
DSH = NH * HD          # 256 attention dims per core
FSH = FF // 4          # 1024 FFN rows per core
NCORES = 8

F32 = mybir.dt.float32
BF16 = mybir.dt.bfloat16
AF = mybir.ActivationFunctionType
ALU = mybir.AluOpType
bf16 = ml_dtypes.bfloat16


def _body(tc, aps, gelu_func):
    nc = tc.nc
    x_d = aps["x"]
    wqkv_d = aps["wqkv"]
    bqkv_d = aps["bqkv"]
    wo_d = aps["wo"]
    w1_d = aps["w1"]
    b1_d = aps["b1p"]
    w2_d = aps["w2"]
    cos_d = aps["cosr"]
    sin_d = aps["sinr"]
    mask_d = aps["maskd"]
    out_d = aps["outp"].rearrange("(c p) s -> c p s", p=P)   # [8, 128, 2048]

    with (
        tc.tile_pool(name="const", bufs=1) as const,
        tc.tile_pool(name="big", bufs=1) as big,
        tc.tile_pool(name="xp", bufs=3) as xp,
        tc.tile_pool(name="statp", bufs=4) as statp,
        tc.tile_pool(name="xhp", bufs=4) as xhp,
        tc.tile_pool(name="rotp", bufs=3) as rotp,
        tc.tile_pool(name="wstp", bufs=6) as wstp,
        tc.tile_pool(name="ptp", bufs=3) as ptp,
        tc.tile_pool(name="sump", bufs=2) as sump,
        tc.tile_pool(name="obp", bufs=4) as obp,
    ):
        # ---- persistent SBUF ----
        wqkv_sb = const.tile([P, DC, 3 * DSH], BF16)
        nc.sync.dma_start(wqkv_sb[:], wqkv_d)
        bqkv_sb = const.tile([P, 3 * DSH], F32)
        nc.sync.dma_start(bqkv_sb[:], bqkv_d)
        wo_sb = const.tile([P, 2, D], BF16)
        nc.sync.dma_start(wo_sb[:], wo_d)
        b1_sb = const.tile([P, DC], F32)
        nc.sync.dma_start(b1_sb[:], b1_d)
        cos_sb = const.tile([P, NT, RH], BF16)
        nc.sync.dma_start(cos_sb[:], cos_d)
        sin_sb = const.tile([P, NT, RH], BF16)
        nc.sync.dma_start(sin_sb[:], sin_d)
        mask_sb = const.tile([P, P], BF16)
        nc.sync.dma_start(mask_sb[:], mask_d)
        eps_sb = const.tile([P, 1], F32)
        nc.vector.memset(eps_sb[:], EPS)
        ones_sb = const.tile([1, HD], BF16)
        nc.vector.memset(ones_sb[:], 1.0)

        xhatT = [big.tile([P, DC, S // 2], BF16, name=f"xhatT{i}")
                 for i in range(2)]          # xhat transposed [d, s], s-halves
        qk = big.tile([P, NT, 2 * DSH], BF16)       # q,k token-major
        vp = big.tile([P, NT, NH, HD + 2], BF16)    # v token-major + ones col
        qe = big.tile([P, 2, S], BF16)              # q e-major
        ke = big.tile([P, 2, S], BF16)              # k e-major
        ot = big.tile([P, 2, S], BF16)              # attn out (normalized), e-major
        hid = big.tile([P, DC, S], BF16)            # ffn hidden, f-major

        nc.vector.memset(vp[:, :, :, HD:HD + 1], 1.0)

        # DRAM staging for the big bf16 transposes (token-major -> dim-major)
        stg = tc.alloc_tile_pool(name="stg", bufs=1, space="DRAM")
        xh_dram = stg.tile([S, D], BF16)
        qk_dram = stg.tile([S, 2 * DSH], BF16)
        ff2_dram = stg.tile([D, S], BF16)

        # ---- Stage A: LayerNorm stats + xhat (bf16), stage via DRAM ----
        for t in range(NT):
            x_t = xp.tile([P, D], F32, tag="xt")
            nc.sync.dma_start(x_t[:], x_d[ts(t, P), :])
            st = statp.tile([P, 2, 6], F32, tag="st")
            xr = x_t[:].rearrange("p (a f) -> p a f", f=512)
            for sg in range(2):
                nc.vector.bn_stats(st[:, sg, :], xr[:, sg, :])
            mv = statp.tile([P, 2], F32, tag="mv")
            nc.vector.bn_aggr(mv[:], st[:])
            std = statp.tile([P, 1], F32, tag="sd")
            nc.scalar.activation(std[:], mv[:, 1:2], AF.Sqrt, bias=eps_sb[:])
            rstd = statp.tile([P, 1], F32, tag="rs")
            nc.vector.reciprocal(rstd[:], std[:])
            mneg = statp.tile([P, 1], F32, tag="mn")
            nc.vector.scalar_tensor_tensor(
                out=mneg[:], in0=mv[:, 0:1], scalar=-1.0, in1=rstd[:],
                op0=ALU.mult, op1=ALU.mult)
            xh = xhp.tile([P, D], BF16, tag="xh")
            nc.scalar.activation(xh[:], x_t[:], AF.Identity,
                                 bias=mneg[:], scale=rstd[:])
            nc.sync.dma_start(xh_dram[ts(t, P), :], xh[:])
            if t % (NT // 2) == NT // 2 - 1:
                half = t // (NT // 2)
                for c in range(DC):
                    nc.sync.dma_start_transpose(
                        xhatT[half][:, c, :],
                        xh_dram[ds(half * (S // 2), S // 2), ds(P * c, P)])

        # ---- Stage B: QKV projection + FFN-1/GELU (one PE-dense phase;
        # GELU runs here so the ACT table never alternates with Exp) ----
        with (
            tc.tile_pool(name="qaps", bufs=2, space="PSUM") as qaps,
            tc.tile_pool(name="qbps", bufs=2, space="PSUM") as qbps,
            tc.tile_pool(name="ff1ps", bufs=4, space="PSUM") as ff1ps,
        ):
            for t in range(NT):
                psa = qaps.tile([P, 512], F32, tag="qa")
                psb = qbps.tile([P, 256], F32, tag="qb")
                for c in range(DC):
                    l = xhatT[t // (NT // 2)][:, c, ts(t % (NT // 2), P)]
                    nc.tensor.matmul(psa[:], lhsT=l, rhs=wqkv_sb[:, c, 0:512],
                                     start=(c == 0), stop=(c == DC - 1))
                    nc.tensor.matmul(psb[:], lhsT=l, rhs=wqkv_sb[:, c, 512:768],
                                     start=(c == 0), stop=(c == DC - 1))
                nc.vector.tensor_tensor(out=qk[:, t, :], in0=psa[:],
                                        in1=bqkv_sb[:, 0:512], op=ALU.add)
                nc.vector.tensor_tensor(
                    out=vp[:, t, :, 0:HD],
                    in0=psb[:].rearrange("p (h e) -> p h e", h=NH),
                    in1=bqkv_sb[:, 512:768].rearrange("p (h e) -> p h e", h=NH),
                    op=ALU.add)

            # rotary on q and k (token-major, in place)
            cosb = cos_sb[:].unsqueeze(2).to_broadcast([P, NT, NH, RH])
            sinb = sin_sb[:].unsqueeze(2).to_broadcast([P, NT, NH, RH])
            for part in range(2):   # 0: q, 1: k
                sl = qk[:, :, ds(DSH * part, DSH)].rearrange(
                    "p t (h e) -> p t h e", h=NH)
                x1 = sl[:, :, :, 0:RH]
                x2 = sl[:, :, :, RH:ROT]
                t1 = rotp.tile([P, NT, NH, RH], BF16, tag="rt")
                t2 = rotp.tile([P, NT, NH, RH], BF16, tag="rt")
                t3 = rotp.tile([P, NT, NH, RH], BF16, tag="rt")
                nc.vector.tensor_tensor(out=t1[:], in0=x1, in1=cosb, op=ALU.mult)
                nc.vector.tensor_tensor(out=t2[:], in0=x2, in1=sinb, op=ALU.mult)
                nc.vector.tensor_tensor(out=t1[:], in0=t1[:], in1=t2[:],
                                        op=ALU.subtract)
                nc.vector.tensor_tensor(out=t2[:], in0=x1, in1=sinb, op=ALU.mult)
                nc.vector.tensor_tensor(out=t3[:], in0=x2, in1=cosb, op=ALU.mult)
                nc.vector.tensor_tensor(out=t2[:], in0=t2[:], in1=t3[:],
                                        op=ALU.add)
                nc.vector.tensor_copy(out=x1, in_=t1[:])
                nc.vector.tensor_copy(out=x2, in_=t2[:])

            # transpose q, k to e-major (via DRAM staging)
            for t in range(NT):
                nc.sync.dma_start(qk_dram[ts(t, P), :], qk[:, t, :])
            for c in range(2):
                nc.sync.dma_start_transpose(qe[:, c, :],
                                            qk_dram[:, ds(P * c, P)])
                nc.sync.dma_start_transpose(ke[:, c, :],
                                            qk_dram[:, ds(DSH + P * c, P)])

            # FFN first matmul + GELU
            for ft in range(DC):
                w1t = wstp.tile([P, DC, P], BF16, tag="wst", name=f"w1t_{ft}")
                nc.scalar.dma_start(w1t[:], w1_d[:, ft])
                pss = [ff1ps.tile([P, 512], F32, tag="ff1",
                                  name=f"ff1_{ft}_{i}") for i in range(4)]
                for c in range(DC):
                    for sc in range(4):
                        nc.tensor.matmul(
                            pss[sc][:], lhsT=w1t[:, c, :],
                            rhs=xhatT[sc // 2][:, c, ds(512 * (sc % 2), 512)],
                            start=(c == 0), stop=(c == DC - 1))
                for sc in range(4):
                    nc.scalar.activation(hid[:, ft, ds(512 * sc, 512)],
                                         pss[sc][:], gelu_func,
                                         bias=b1_sb[:, ft:ft + 1])

        # ---- Stage C: attention (ACT-bound exp) interleaved with FFN-2
        # matmuls (pure PE+DVE, partials staged to DRAM in bf16) ----
        ff2_deck = [[0, 1], [2, 3], [4, 5], [6, 7], []]
        with (
            tc.tile_pool(name="scps", bufs=3, space="PSUM") as scps,
            tc.tile_pool(name="ovps", bufs=4, space="PSUM") as ovps,
            tc.tile_pool(name="f2ps", bufs=1, space="PSUM") as f2ps,
        ):
            def emit_ff2(et):
                w2t = wstp.tile([P, DC, P], BF16, tag="wst", name=f"w2t_{et}")
                nc.scalar.dma_start(w2t[:], w2_d[:, et])
                for sc in range(4):
                    ps = f2ps.tile([P, 512], F32, tag="f2",
                                   name=f"f2_{et}_{sc}")
                    for c in range(DC):
                        nc.tensor.matmul(ps[:], lhsT=w2t[:, c, :],
                                         rhs=hid[:, c, ds(512 * sc, 512)],
                                         start=(c == 0), stop=(c == DC - 1))
                    ob2 = obp.tile([P, 512], BF16, tag="ob2")
                    nc.vector.tensor_copy(out=ob2[:], in_=ps[:])
                    nc.sync.dma_start(
                        ff2_dram[ds(P * et, P), ds(512 * sc, 512)], ob2[:])

            def emit_head(h):
                base = HD * (h % 2)
                cix = h // 2
                ov = [ovps.tile([HD + 1, 512], F32, tag="ov",
                                name=f"ov_{h}_{i}") for i in range(4)]
                for i in range(NT):
                    ncols = S - P * i
                    pt = ptp.tile([P, S], BF16, tag="pt")
                    nb = (ncols + 511) // 512
                    for bi in range(nb):
                        w = min(512, ncols - 512 * bi)
                        ps = scps.tile([P, 512], F32, tag="sc")
                        nc.tensor.matmul(
                            ps[:, :w],
                            lhsT=ke[base:base + HD, cix, ts(i, P)],
                            rhs=qe[base:base + HD, cix, ds(P * i + 512 * bi, w)],
                            start=True, stop=True)
                        nc.scalar.activation(pt[:, ds(512 * bi, w)], ps[:, :w],
                                             AF.Exp, scale=0.125)
                    nc.vector.tensor_tensor(out=pt[:, 0:P], in0=pt[:, 0:P],
                                            in1=mask_sb[:], op=ALU.mult)
                    for sc in range(i // 4, 4):
                        lo = max(512 * sc, P * i)
                        wid = 512 * (sc + 1) - lo
                        nc.tensor.matmul(
                            ov[sc][:, ds(lo - 512 * sc, wid)],
                            lhsT=vp[:, i, h, 0:HD + 1],
                            rhs=pt[:, ds(lo - P * i, wid)],
                            start=(i == 0), stop=(i == min(NT - 1, 4 * sc + 3)))
                rinvs = []
                for sc in range(4):
                    dst = ot[base:base + HD, cix, ds(512 * sc, 512)]
                    nc.vector.tensor_copy(out=dst, in_=ov[sc][0:HD, :])
                    sume = sump.tile([1, 512], F32, tag="se",
                                     name=f"se_{h}_{sc}")
                    nc.vector.tensor_copy(out=sume[:], in_=ov[sc][HD:HD + 1, :])
                    rinv = sump.tile([1, 512], F32, tag="ri",
                                     name=f"ri_{h}_{sc}")
                    nc.vector.reciprocal_approx_fast(out=rinv[:], in_=sume[:])
                    rinv_bf = sump.tile([1, 512], BF16, tag="rib",
                                        name=f"rib_{h}_{sc}")
                    nc.vector.tensor_copy(out=rinv_bf[:], in_=rinv[:])
                    rinvs.append(rinv_bf)
                for sc in range(4):
                    rbp = ovps.tile([HD + 1, 512], F32, tag="ov",
                                    name=f"rb_{h}_{sc}")
                    nc.tensor.matmul(rbp[0:HD, :], lhsT=ones_sb[:],
                                     rhs=rinvs[sc][:], start=True, stop=True)
                    dst = ot[base:base + HD, cix, ds(512 * sc, 512)]
                    nc.vector.tensor_tensor(out=dst, in0=dst, in1=rbp[0:HD, :],
                                            op=ALU.mult)

            for h in range(NH):
                emit_head(h)
                for et in ff2_deck[h]:
                    emit_ff2(et)

        # ---- Stage D: attn-out projection + add staged FFN-2 partials ----
        with tc.tile_pool(name="wops", bufs=4, space="PSUM") as wops:
            for et in range(DC):
                for sc in range(4):
                    po = wops.tile([P, 512], F32, tag="wo")
                    for c in range(2):
                        nc.tensor.matmul(po[:], lhsT=wo_sb[:, c, ts(et, P)],
                                         rhs=ot[:, c, ds(512 * sc, 512)],
                                         start=(c == 0), stop=(c == 1))
                    f2b = obp.tile([P, 512], BF16, tag="f2b")
                    nc.scalar.dma_start(
                        f2b[:], ff2_dram[ds(P * et, P), ds(512 * sc, 512)])
                    ob = obp.tile([P, 512], F32, tag="ob")
                    nc.vector.tensor_tensor(out=ob[:], in0=po[:], in1=f2b[:],
                                            op=ALU.add)
                    nc.sync.dma_start(out_d[et][:, ds(512 * sc, 512)], ob[:])
        stg.release()


def build(gelu_func=None):
    if gelu_func is None:
        gelu_func = AF.Gelu
    nc = bacc.Bacc("TRN2", target_bir_lowering=False, debug=False,
                   enable_asserts=True, num_devices=NCORES)
    aps = {}

    def din(name, shape, dtype):
        aps[name] = nc.dram_tensor(name, list(shape), dtype,
                                   kind="ExternalInput").ap()

    din("x", (S, D), F32)
    din("wqkv", (P, DC, 3 * DSH), BF16)
    din("bqkv", (P, 3 * DSH), F32)
    din("wo", (P, 2, D), BF16)
    din("w1", (P, DC, DC, P), BF16)
    din("b1p", (P, DC), F32)
    din("w2", (P, DC, DC, P), BF16)
    din("cosr", (P, NT, RH), BF16)
    din("sinr", (P, NT, RH), BF16)
    din("maskd", (P, P), BF16)
    aps["outp"] = nc.dram_tensor("outp", [D, S], F32,
                                 kind="ExternalOutput").ap()

    with tile.TileContext(nc) as tc:
        _body(tc, aps, gelu_func)
    nc.compile()
    return nc


def make_in_maps(inputs):
    x = np.asarray(inputs["x"], np.float32)
    Wqkv = np.asarray(inputs["W_qkv"], np.float32)
    b_qkv = np.asarray(inputs["b_qkv"], np.float32)
    Wo = np.asarray(inputs["W_o"], np.float32)
    ln1w = np.asarray(inputs["ln1_w"], np.float32)
    ln1b = np.asarray(inputs["ln1_b"], np.float32)
    ln2w = np.asarray(inputs["ln2_w"], np.float32)
    ln2b = np.asarray(inputs["ln2_b"], np.float32)
    W1 = np.asarray(inputs["W1"], np.float32)
    b1 = np.asarray(inputs["b1"], np.float32)
    W2 = np.asarray(inputs["W2"], np.float32)
    freqs = np.asarray(inputs["freqs_cis"], np.float32)

    cos = freqs[0, 0, :, :, 0]
    sin = freqs[0, 0, :, :, 1]
    cosr = np.ascontiguousarray(
        cos.reshape(NT, P, RH).transpose(1, 0, 2)).astype(bf16)
    sinr = np.ascontiguousarray(
        sin.reshape(NT, P, RH).transpose(1, 0, 2)).astype(bf16)
    kq = np.arange(P)
    maskd = (kq[:, None] <= kq[None, :]).astype(bf16)

    in_maps = []
    for core in range(NCORES):
        b = core // 4
        r = core % 4
        sl = slice(256 * r, 256 * r + 256)
        Ws = np.concatenate([Wqkv[0:D][sl], Wqkv[D:2 * D][sl],
                             Wqkv[2 * D:3 * D][sl]], 0)          # [768, 1024]
        bq = np.concatenate([b_qkv[0:D][sl], b_qkv[D:2 * D][sl],
                             b_qkv[2 * D:3 * D][sl]], 0)
        Wsp = Ws * ln1w[None, :]
        bqp = (bq + Ws @ ln1b).astype(np.float32)
        wqkv_l = np.ascontiguousarray(
            Wsp.T.reshape(DC, P, 3 * DSH).transpose(1, 0, 2)).astype(bf16)
        bqkv_l = np.ascontiguousarray(
            np.broadcast_to(bqp[None, :], (P, 3 * DSH))).astype(np.float32)
        Wos = Wo[:, sl]                                           # [1024, 256]
        wo_l = np.ascontiguousarray(
            Wos.T.reshape(2, P, D).transpose(1, 0, 2)).astype(bf16)
        W1s = W1[FSH * r: FSH * (r + 1)]                          # [1024, 1024]
        W1p = W1s * ln2w[None, :]
        b1p = (b1[FSH * r: FSH * (r + 1)] + W1s @ ln2b).astype(np.float32)
        w1_l = np.ascontiguousarray(
            W1p.reshape(DC, P, DC, P).transpose(3, 0, 2, 1)).astype(bf16)
        b1_l = np.ascontiguousarray(b1p.reshape(DC, P).T).astype(np.float32)
        W2s = W2[:, FSH * r: FSH * (r + 1)]                       # [1024, 1024]
        w2_l = np.ascontiguousarray(
            W2s.reshape(DC, P, DC, P).transpose(3, 0, 2, 1)).astype(bf16)
        in_maps.append(dict(
            x=np.ascontiguousarray(x[b]), wqkv=wqkv_l, bqkv=bqkv_l, wo=wo_l,
            w1=w1_l, b1p=b1_l, w2=w2_l, cosr=cosr, sinr=sinr, maskd=maskd))
    return in_maps


def gather(inputs, results):
    x = np.asarray(inputs["x"], np.float32)
    bias = (np.asarray(inputs["b_o"], np.float32)
            + np.asarray(inputs["b2"], np.float32))
    outs = [np.asarray(res["outp"], np.float32) for res in results]
    out = np.empty((B, S, D), np.float32)
    for b in range(B):
        acc = outs[4 * b] + outs[4 * b + 1] + outs[4 * b + 2] + outs[4 * b + 3]
        out[b] = x[b] + acc.T + bias[None, :]
    return out


_CACHE = {}


def kernel(**inputs):
    if "nc" not in _CACHE:
        _CACHE["nc"] = build()
    nc = _CACHE["nc"]
    in_maps = make_in_maps(inputs)
    res = bass_utils.run_bass_kernel_spmd(nc, in_maps,
                                          core_ids=list(range(NCORES)))
    return gather(inputs, res.results)
